# revision 11
# baseline (speedup 1.0000x reference)
"""Trainium2 Bass kernel for exponential smoothing (EMA over time).

Math: out[b,t,h,d] = w_h^{t+1} v0[h,d] + sum_{j<=t} (1-w_h) w_h^{t-j} x[b,j,h,d]
(w = sigmoid(smoothing_weight)), i.e. the scan s_t = w s_{t-1} + (1-w) x_t with
s_{-1} = v0.

Kernel strategy (per core, data-parallel over batch: 16 batches / 8 cores):
  - Time is processed in chunks of C=127. Each chunk is one set of 8 per-head
    matmuls: rhs = [carry_row; x rows] (128 x 64 per head), lhsT packs the
    decay column w^{p+1} (for the carry) on top of the lower-triangular
    smoothing weights (1-w) w^{p-j}. The *corrected* last output row of a
    chunk IS the carry for the next chunk, so cross-chunk propagation is a
    single [1,512] PSUM->SBUF row copy per chunk.
  - 4096 = 32*127 + 32: 32 full chunks + one 32-row tail chunk per batch.
  - Input tiles are loaded contiguously ([128, 4*512] per 4-chunk group,
    ~1MB per dma_start) with an f32->bf16 cast on the SWDGE path; matmuls run
    in bf16 (fp32 PSUM accumulate), output is evicted to f32 and stored
    contiguously.
"""

import numpy as np

B, T, H, D = 16, 4096, 8, 64
HD = H * D                    # 512
C = 127                       # chunk length (1 row reserved for the carry)
NFULL = T // C                # 32 full chunks
REM = T - NFULL * C           # 32-row tail chunk
GROUPS = NFULL // 4           # 8 groups of 4 chunks (one ~1MB DMA each)
NCORES = 8
BPC = B // NCORES             # batches per core

COMPUTE_DTYPE = "bf16"        # "bf16" | "fp32"

_cache = {}


def _host_constants(smoothing_weight, v0, np_cdtype):
    """Parameter-derived constants, computed in fp64 on host."""
    w = 1.0 / (1.0 + np.exp(-smoothing_weight.astype(np.float64)))  # [H,1]
    w = w[:, 0]

    def make_lhsT(n):
        # [H, n+1, n]; row 0 = w^(p+1) (carry decay), row 1+j = (1-w) w^(p-j)
        lt = np.zeros((H, n + 1, n), dtype=np.float64)
        p = np.arange(n)
        for hh in range(H):
            lt[hh, 0, :] = w[hh] ** (p + 1)
            for j in range(n):
                lt[hh, 1 + j, j:] = (1.0 - w[hh]) * w[hh] ** (p[j:] - j)
        return lt.astype(np_cdtype)

    wt = make_lhsT(C)          # [H, 128, 127]
    # Engine APs must start at 32-aligned partitions, so the carry row (last
    # output row of a chunk) is moved to PSUM partition 0: permute lhsT
    # columns to [last, 0..last-1]; the out-DMA un-permutes.
    wt = np.concatenate([wt[:, :, C - 1:], wt[:, :, :C - 1]], axis=2)
    wt2 = make_lhsT(REM)       # [H, 33, 32] (tail: no carry out, unpermuted)
    # [K, H, M] layout so the on-chip weight DMA is contiguous per partition
    wt = np.ascontiguousarray(wt.transpose(1, 0, 2))    # [128, 8, 127]
    wt2 = np.ascontiguousarray(wt2.transpose(1, 0, 2))  # [33, 8, 32]
    v0row = v0.reshape(1, HD).astype(np_cdtype)   # [1, 512]
    return wt, wt2, v0row


def _build_program(cdtype_name):
    import concourse.bass as bass
    import concourse.tile as tile
    from concourse import bacc, mybir

    cdtype = mybir.dt.bfloat16 if cdtype_name == "bf16" else mybir.dt.float32
    f32 = mybir.dt.float32

    nc = bacc.Bacc("TRN2", target_bir_lowering=False, debug=False,
                   num_devices=NCORES)

    x_d = nc.dram_tensor("x", [BPC, T, HD], f32, kind="ExternalInput").ap()
    wt_d = nc.dram_tensor("wt", [C + 1, H, C], cdtype, kind="ExternalInput").ap()
    wt2_d = nc.dram_tensor("wt2", [REM + 1, H, REM], cdtype,
                           kind="ExternalInput").ap()
    v0_d = nc.dram_tensor("v0r", [1, HD], cdtype, kind="ExternalInput").ap()
    out_d = nc.dram_tensor("out", [BPC, T, HD], f32, kind="ExternalOutput").ap()

    from contextlib import ExitStack
    with tile.TileContext(nc) as tc, ExitStack() as ctx:
        consts = ctx.enter_context(tc.tile_pool(name="consts", bufs=1))
        in_pool = ctx.enter_context(tc.tile_pool(name="inp", bufs=6))
        in2_pool = ctx.enter_context(tc.tile_pool(name="inp2", bufs=2))
        out_pool = ctx.enter_context(tc.tile_pool(name="outp", bufs=6))
        out2_pool = ctx.enter_context(tc.tile_pool(name="outp2", bufs=2))
        psum_pool = ctx.enter_context(tc.tile_pool(name="psum", bufs=6,
                                                   space="PSUM"))
        psum2_pool = ctx.enter_context(tc.tile_pool(name="psum2", bufs=2,
                                                    space="PSUM"))
        if True:
            # --- constants ---
            wt_s = consts.tile([C + 1, H, C], cdtype)       # [128, 8, 127]
            nc.sync.dma_start(out=wt_s[:], in_=wt_d)
            wt2_s = consts.tile([REM + 1, H, REM], cdtype)  # [33, 8, 32]
            nc.sync.dma_start(out=wt2_s[:], in_=wt2_d)
            v0_s = consts.tile([1, HD], cdtype)
            nc.sync.dma_start(out=v0_s[:], in_=v0_d[:])

            # --- tile handles ---
            in_tiles = {(b, g): in_pool.tile([C + 1, 4, HD], cdtype, tag="in", name=f"in_{b}_{g}")
                        for b in range(BPC) for g in range(GROUPS)}
            in2_tiles = {b: in2_pool.tile([REM + 1, HD], cdtype, tag="in2", name=f"in2_{b}")
                         for b in range(BPC)}
            stage_pool = ctx.enter_context(tc.tile_pool(name="stg", bufs=3))
            stage2_pool = ctx.enter_context(tc.tile_pool(name="stg2", bufs=2))

            def load_group(b, g):
                # Must be emitted BEFORE any carry copy that targets this
                # tile's row 0: the full-tile cast writes garbage there and
                # Tile orders same-region writes by program order.
                it = in_tiles[(b, g)]
                src = x_d[b, 4 * C * g: 4 * C * (g + 1), :] \
                    .rearrange("(k p) c -> p k c", p=C)
                if cdtype_name == "bf16":
                    # HWDGE f32 load into staging rows 1..127, then a
                    # full-tile cast on the (otherwise idle) GPSIMD engine;
                    # row 0 holds dummy data (engine APs must start at an
                    # aligned partition, so the cast covers the full tile)
                    # until the carry copy overwrites it.
                    stg = stage_pool.tile([C + 1, 4, HD], f32, tag="stg")
                    nc.sync.dma_start(out=stg[1:C + 1, :, :], in_=src)
                    r0 = 4 * C * g - 1 if g > 0 else 0
                    dummy = x_d[b, r0: r0 + 4 * C, :] \
                        .rearrange("(k p) c -> p k c", p=C)[0:1, :, :]
                    nc.sync.dma_start(out=stg[0:1, :, :], in_=dummy)
                    nc.gpsimd.tensor_copy(it[:, :, :], stg[:, :, :])
                else:
                    nc.sync.dma_start(out=it[1:C + 1, :, :], in_=src)
                if g == 0:
                    # carry-in for chunk 0 is v0
                    nc.vector.tensor_copy(it[0:1, 0, :], v0_s[:])

            for b in range(BPC):
                load_group(b, 0)
            for g in range(GROUPS):
                for b in range(BPC):
                    if g + 1 < GROUPS:
                        load_group(b, g + 1)
                    it = in_tiles[(b, g)]

                    ot = out_pool.tile([C, 4, HD], mybir.dt.float32, tag="out")
                    for k in range(4):
                        ps = psum_pool.tile([C, HD], mybir.dt.float32, tag="ps")
                        for hh in range(H):
                            nc.tensor.matmul(
                                out=ps[:, hh * D:(hh + 1) * D],
                                lhsT=wt_s[:, hh, :],
                                rhs=it[:, k, hh * D:(hh + 1) * D],
                                start=True, stop=True,
                            )
                        # carry: corrected last row (at PSUM partition 0 via
                        # the lhsT permutation) -> next chunk's rhs row 0
                        if k < 3:
                            dst = in_tiles[(b, g)][0:1, k + 1, :]
                        elif g < GROUPS - 1:
                            dst = in_tiles[(b, g + 1)][0:1, 0, :]
                        else:
                            dst = in2_tiles[b][0:1, :]
                        nc.vector.tensor_copy(dst, ps[0:1, :])
                        # evict to f32 output staging (rows stay permuted)
                        nc.scalar.copy(ot[:, k, :], ps[:, :])
                    dstv = out_d[b, 4 * C * g: 4 * C * (g + 1), :] \
                        .rearrange("(k p) c -> p k c", p=C)
                    # un-permute: ot partition 0 = chunk's last time row
                    nc.sync.dma_start(out=dstv[0:C - 1, :, :],
                                      in_=ot[1:C, :, :])
                    nc.sync.dma_start(out=dstv[C - 1:C, :, :],
                                      in_=ot[0:1, :, :])

            # --- tail chunk (32 rows) per batch ---
            for b in range(BPC):
                it2 = in2_tiles[b]
                src = x_d[b, NFULL * C:, :]                  # [32, 512]
                if cdtype_name == "bf16":
                    nc.gpsimd.dma_start(out=it2[1:REM + 1, :], in_=src)
                else:
                    nc.sync.dma_start(out=it2[1:REM + 1, :], in_=src)
                ps2 = psum2_pool.tile([REM, HD], mybir.dt.float32, tag="ps2")
                for hh in range(H):
                    nc.tensor.matmul(
                        out=ps2[:, hh * D:(hh + 1) * D],
                        lhsT=wt2_s[:, hh, :],
                        rhs=it2[:, hh * D:(hh + 1) * D],
                        start=True, stop=True,
                    )
                ot2 = out2_pool.tile([REM, HD], mybir.dt.float32, tag="out2")
                nc.scalar.copy(ot2[:], ps2[:])
                nc.sync.dma_start(out=out_d[b, NFULL * C:, :], in_=ot2[:])

    nc.compile()
    return nc


def _get_program():
    key = COMPUTE_DTYPE
    if key not in _cache:
        _cache[key] = _build_program(key)
    return _cache[key]


def kernel(values, smoothing_weight, v0):
    import ml_dtypes
    from concourse.bass_utils import run_bass_kernel_spmd

    np_cdtype = ml_dtypes.bfloat16 if COMPUTE_DTYPE == "bf16" else np.float32
    wt, wt2, v0row = _host_constants(smoothing_weight, v0, np_cdtype)

    nc = _get_program()
    x = np.ascontiguousarray(values.astype(np.float32)
                             .reshape(B, T, HD))
    in_maps = []
    for core in range(NCORES):
        shard = np.ascontiguousarray(x[core * BPC:(core + 1) * BPC])
        in_maps.append({"x": shard, "wt": wt, "wt2": wt2, "v0r": v0row})

    res = run_bass_kernel_spmd(nc, in_maps, list(range(NCORES)))
    outs = [res.results[i]["out"].reshape(BPC, T, H, D)
            for i in range(NCORES)]
    return np.concatenate(outs, axis=0).astype(np.float32)


# revision 12
# speedup vs baseline: 3.7672x; 3.7672x over previous
"""Trainium2 Bass kernel for exponential smoothing (EMA over time).

Math: out[b,t,h,d] = w_h^{t+1} v0[h,d] + sum_{j<=t} (1-w_h) w_h^{t-j} x[b,j,h,d]
(w = sigmoid(smoothing_weight)), i.e. the scan s_t = w s_{t-1} + (1-w) x_t with
s_{-1} = v0.

Kernel strategy (per core, data-parallel over batch: 16 batches / 8 cores):
  - Time is processed in chunks of C=127. Each chunk is one set of 8 per-head
    matmuls: rhs = [carry_row; x rows] (128 x 64 per head), lhsT packs the
    decay column w^{p+1} (for the carry) on top of the lower-triangular
    smoothing weights (1-w) w^{p-j}. The *corrected* last output row of a
    chunk IS the carry for the next chunk, so cross-chunk propagation is a
    single [1,512] PSUM->SBUF row copy per chunk.
  - 4096 = 32*127 + 32: 32 full chunks + one 32-row tail chunk per batch.
  - Input tiles are loaded contiguously ([128, 4*512] per 4-chunk group,
    ~1MB per dma_start) with an f32->bf16 cast on the SWDGE path; matmuls run
    in bf16 (fp32 PSUM accumulate), output is evicted to f32 and stored
    contiguously.
"""

import numpy as np

B, T, H, D = 16, 4096, 8, 64
HD = H * D                    # 512
C = 127                       # chunk length (1 row reserved for the carry)
NFULL = T // C                # 32 full chunks
REM = T - NFULL * C           # 32-row tail chunk
GROUPS = NFULL // 4           # 8 groups of 4 chunks (one ~1MB DMA each)
NCORES = 8
BPC = B // NCORES             # batches per core

COMPUTE_DTYPE = "bf16"        # "bf16" | "fp32"

_cache = {}


def _host_constants(smoothing_weight, v0, np_cdtype):
    """Parameter-derived constants, computed in fp64 on host."""
    w = 1.0 / (1.0 + np.exp(-smoothing_weight.astype(np.float64)))  # [H,1]
    w = w[:, 0]

    def make_lhsT(n):
        # [H, n+1, n]; row 0 = w^(p+1) (carry decay), row 1+j = (1-w) w^(p-j)
        lt = np.zeros((H, n + 1, n), dtype=np.float64)
        p = np.arange(n)
        for hh in range(H):
            lt[hh, 0, :] = w[hh] ** (p + 1)
            for j in range(n):
                lt[hh, 1 + j, j:] = (1.0 - w[hh]) * w[hh] ** (p[j:] - j)
        return lt.astype(np_cdtype)

    wt = make_lhsT(C)          # [H, 128, 127]
    # Engine APs must start at 32-aligned partitions, so the carry row (last
    # output row of a chunk) is moved to PSUM partition 0: permute lhsT
    # columns to [last, 0..last-1]; the out-DMA un-permutes.
    wt = np.concatenate([wt[:, :, C - 1:], wt[:, :, :C - 1]], axis=2)
    wt2 = make_lhsT(REM)       # [H, 33, 32] (tail: no carry out, unpermuted)
    # [K, H, M] layout so the on-chip weight DMA is contiguous per partition
    wt = np.ascontiguousarray(wt.transpose(1, 0, 2))    # [128, 8, 127]
    wt2 = np.ascontiguousarray(wt2.transpose(1, 0, 2))  # [33, 8, 32]
    v0row = v0.reshape(1, HD).astype(np_cdtype)   # [1, 512]
    return wt, wt2, v0row


def _build_program(cdtype_name):
    import concourse.bass as bass
    import concourse.tile as tile
    from concourse import bacc, mybir

    cdtype = mybir.dt.bfloat16 if cdtype_name == "bf16" else mybir.dt.float32
    f32 = mybir.dt.float32

    nc = bacc.Bacc("TRN2", target_bir_lowering=False, debug=False,
                   num_devices=NCORES)

    x_d = nc.dram_tensor("x", [BPC, T, HD], f32, kind="ExternalInput").ap()
    wt_d = nc.dram_tensor("wt", [C + 1, H, C], cdtype, kind="ExternalInput").ap()
    wt2_d = nc.dram_tensor("wt2", [REM + 1, H, REM], cdtype,
                           kind="ExternalInput").ap()
    v0_d = nc.dram_tensor("v0r", [1, HD], cdtype, kind="ExternalInput").ap()
    out_d = nc.dram_tensor("out", [BPC, T, HD], f32, kind="ExternalOutput").ap()

    from contextlib import ExitStack
    with tile.TileContext(nc) as tc, ExitStack() as ctx:
        consts = ctx.enter_context(tc.tile_pool(name="consts", bufs=1))
        in_pool = ctx.enter_context(tc.tile_pool(name="inp", bufs=6))
        in2_pool = ctx.enter_context(tc.tile_pool(name="inp2", bufs=2))
        out_pool = ctx.enter_context(tc.tile_pool(name="outp", bufs=6))
        out2_pool = ctx.enter_context(tc.tile_pool(name="outp2", bufs=2))
        psum_pool = ctx.enter_context(tc.tile_pool(name="psum", bufs=6,
                                                   space="PSUM"))
        psum2_pool = ctx.enter_context(tc.tile_pool(name="psum2", bufs=2,
                                                    space="PSUM"))
        if True:
            # --- constants ---
            wt_s = consts.tile([C + 1, H, C], cdtype)       # [128, 8, 127]
            nc.sync.dma_start(out=wt_s[:], in_=wt_d)
            wt2_s = consts.tile([REM + 1, H, REM], cdtype)  # [33, 8, 32]
            nc.sync.dma_start(out=wt2_s[:], in_=wt2_d)
            v0_s = consts.tile([1, HD], cdtype)
            nc.sync.dma_start(out=v0_s[:], in_=v0_d[:])

            # --- tile handles ---
            in_tiles = {(b, g): in_pool.tile([C + 1, 4, HD], cdtype, tag="in", name=f"in_{b}_{g}")
                        for b in range(BPC) for g in range(GROUPS)}
            in2_tiles = {b: in2_pool.tile([REM + 1, HD], cdtype, tag="in2", name=f"in2_{b}")
                         for b in range(BPC)}
            stage_pool = ctx.enter_context(tc.tile_pool(name="stg", bufs=3))
            stage2_pool = ctx.enter_context(tc.tile_pool(name="stg2", bufs=2))

            def load_group(b, g):
                # Must be emitted BEFORE any carry copy that targets this
                # tile's row 0: the full-tile cast writes garbage there and
                # Tile orders same-region writes by program order.
                #
                # DMAs whose SBUF side is not aligned to the 8-partition
                # AXI port groups serialize onto a single SDMA engine
                # (measured 24 GB/s), so the load covers all 128 partitions
                # with an overlapping source AP: row 0 receives the chunk's
                # preceding time row (real data, replaced by the carry copy
                # before any matmul reads it).
                it = in_tiles[(b, g)]
                xb = x_d[b]
                stg = stage_pool.tile([C + 1, 4, HD], f32, tag="stg")
                if g > 0:
                    src = bass.AP(
                        tensor=xb.tensor,
                        offset=xb.offset + (4 * C * g - 1) * HD,
                        ap=[[HD, C + 1], [C * HD, 4], [1, HD]],
                    )
                    nc.sync.dma_start(out=stg[:, :, :], in_=src)
                else:
                    # chunks 1..3 via the overlap trick; chunk 0 has no
                    # predecessor row, so its x rows load misaligned (254KB
                    # once per batch, hidden in startup) plus one junk row.
                    src = bass.AP(
                        tensor=xb.tensor,
                        offset=xb.offset + (C - 1) * HD,
                        ap=[[HD, C + 1], [C * HD, 3], [1, HD]],
                    )
                    nc.sync.dma_start(out=stg[:, 1:4, :], in_=src)
                    nc.sync.dma_start(out=stg[1:C + 1, 0, :], in_=xb[0:C, :])
                    nc.sync.dma_start(out=stg[0:1, 0, :], in_=xb[0:1, :])
                if cdtype_name == "bf16":
                    nc.gpsimd.tensor_copy(it[:, :, :], stg[:, :, :])
                else:
                    nc.vector.tensor_copy(it[:, :, :], stg[:, :, :])
                if g == 0:
                    # carry-in for chunk 0 is v0
                    nc.vector.tensor_copy(it[0:1, 0, :], v0_s[:])

            for b in range(BPC):
                load_group(b, 0)
            for g in range(GROUPS):
                for b in range(BPC):
                    if g + 1 < GROUPS:
                        load_group(b, g + 1)
                    it = in_tiles[(b, g)]

                    ot = out_pool.tile([C, 4, HD], mybir.dt.float32, tag="out")
                    for k in range(4):
                        ps = psum_pool.tile([C, HD], mybir.dt.float32, tag="ps")
                        for hh in range(H):
                            nc.tensor.matmul(
                                out=ps[:, hh * D:(hh + 1) * D],
                                lhsT=wt_s[:, hh, :],
                                rhs=it[:, k, hh * D:(hh + 1) * D],
                                start=True, stop=True,
                            )
                        # carry: corrected last row (at PSUM partition 0 via
                        # the lhsT permutation) -> next chunk's rhs row 0
                        if k < 3:
                            dst = in_tiles[(b, g)][0:1, k + 1, :]
                        elif g < GROUPS - 1:
                            dst = in_tiles[(b, g + 1)][0:1, 0, :]
                        else:
                            dst = in2_tiles[b][0:1, :]
                        nc.vector.tensor_copy(dst, ps[0:1, :])
                        # evict to f32 output staging (rows stay permuted)
                        nc.scalar.copy(ot[:, k, :], ps[:, :])
                    dstv = out_d[b, 4 * C * g: 4 * C * (g + 1), :] \
                        .rearrange("(k p) c -> p k c", p=C)
                    # un-permute: ot partition 0 = chunk's last time row
                    nc.sync.dma_start(out=dstv[0:C - 1, :, :],
                                      in_=ot[1:C, :, :])
                    nc.sync.dma_start(out=dstv[C - 1:C, :, :],
                                      in_=ot[0:1, :, :])

            # --- tail chunk (32 rows) per batch ---
            for b in range(BPC):
                it2 = in2_tiles[b]
                src = x_d[b, NFULL * C:, :]                  # [32, 512]
                if cdtype_name == "bf16":
                    nc.gpsimd.dma_start(out=it2[1:REM + 1, :], in_=src)
                else:
                    nc.sync.dma_start(out=it2[1:REM + 1, :], in_=src)
                ps2 = psum2_pool.tile([REM, HD], mybir.dt.float32, tag="ps2")
                for hh in range(H):
                    nc.tensor.matmul(
                        out=ps2[:, hh * D:(hh + 1) * D],
                        lhsT=wt2_s[:, hh, :],
                        rhs=it2[:, hh * D:(hh + 1) * D],
                        start=True, stop=True,
                    )
                ot2 = out2_pool.tile([REM, HD], mybir.dt.float32, tag="out2")
                nc.scalar.copy(ot2[:], ps2[:])
                nc.sync.dma_start(out=out_d[b, NFULL * C:, :], in_=ot2[:])

    nc.compile()
    return nc


def _get_program():
    key = COMPUTE_DTYPE
    if key not in _cache:
        _cache[key] = _build_program(key)
    return _cache[key]


def kernel(values, smoothing_weight, v0):
    import ml_dtypes
    from concourse.bass_utils import run_bass_kernel_spmd

    np_cdtype = ml_dtypes.bfloat16 if COMPUTE_DTYPE == "bf16" else np.float32
    wt, wt2, v0row = _host_constants(smoothing_weight, v0, np_cdtype)

    nc = _get_program()
    x = np.ascontiguousarray(values.astype(np.float32)
                             .reshape(B, T, HD))
    in_maps = []
    for core in range(NCORES):
        shard = np.ascontiguousarray(x[core * BPC:(core + 1) * BPC])
        in_maps.append({"x": shard, "wt": wt, "wt2": wt2, "v0r": v0row})

    res = run_bass_kernel_spmd(nc, in_maps, list(range(NCORES)))
    outs = [res.results[i]["out"].reshape(BPC, T, H, D)
            for i in range(NCORES)]
    return np.concatenate(outs, axis=0).astype(np.float32)


# revision 15
# speedup vs baseline: 4.5083x; 1.1967x over previous
"""Trainium2 Bass kernel for exponential smoothing (EMA over time).

Math: out[b,t,h,d] = w_h^{t+1} v0[h,d] + sum_{j<=t} (1-w_h) w_h^{t-j} x[b,j,h,d]
(w = sigmoid(smoothing_weight)), i.e. the scan s_t = w s_{t-1} + (1-w) x_t with
s_{-1} = v0.

Kernel strategy (per core, data-parallel over batch: 16 batches / 8 cores):
  - Time is processed in chunks of C=127. Each chunk is one set of 8 per-head
    matmuls: rhs = [carry_row; x rows] (128 x 64 per head), lhsT packs the
    decay column w^{p+1} (for the carry) on top of the lower-triangular
    smoothing weights (1-w) w^{p-j}. The *corrected* last output row of a
    chunk IS the carry for the next chunk, so cross-chunk propagation is a
    single [1,512] PSUM->SBUF row copy per chunk.
  - 4096 = 32*127 + 32: 32 full chunks + one 32-row tail chunk per batch.
  - Input tiles are loaded contiguously ([128, 4*512] per 4-chunk group,
    ~1MB per dma_start) with an f32->bf16 cast on the SWDGE path; matmuls run
    in bf16 (fp32 PSUM accumulate), output is evicted to f32 and stored
    contiguously.
"""

import numpy as np

B, T, H, D = 16, 4096, 8, 64
HD = H * D                    # 512
C = 127                       # chunk length (1 row reserved for the carry)
NFULL = T // C                # 32 full chunks
REM = T - NFULL * C           # 32-row tail chunk
GROUPS = NFULL // 4           # 8 groups of 4 chunks (one ~1MB DMA each)
NCORES = 8
BPC = B // NCORES             # batches per core

COMPUTE_DTYPE = "bf16"        # "bf16" | "fp32"

_cache = {}


def _host_constants(smoothing_weight, v0, np_cdtype):
    """Parameter-derived constants, computed in fp64 on host."""
    w = 1.0 / (1.0 + np.exp(-smoothing_weight.astype(np.float64)))  # [H,1]
    w = w[:, 0]

    def make_lhsT(n):
        # [H, n+1, n]; row 0 = w^(p+1) (carry decay), row 1+j = (1-w) w^(p-j)
        lt = np.zeros((H, n + 1, n), dtype=np.float64)
        p = np.arange(n)
        for hh in range(H):
            lt[hh, 0, :] = w[hh] ** (p + 1)
            for j in range(n):
                lt[hh, 1 + j, j:] = (1.0 - w[hh]) * w[hh] ** (p[j:] - j)
        return lt.astype(np_cdtype)

    wt = make_lhsT(C)          # [H, 128, 127]
    # Engine APs must start at 32-aligned partitions, so the carry row (last
    # output row of a chunk) is moved to PSUM partition 0: permute lhsT
    # columns to [last, 0..last-1]; the out-DMA un-permutes.
    wt = np.concatenate([wt[:, :, C - 1:], wt[:, :, :C - 1]], axis=2)
    wt2 = make_lhsT(REM)       # [H, 33, 32] (tail: no carry out, unpermuted)
    # pad M to 128 (zero column): Fast Weight Load needs NumWeights == 128;
    # the extra PSUM row is never read
    wt = np.concatenate([wt, np.zeros((H, C + 1, 1), wt.dtype)], axis=2)
    # [K, H, M] layout so the on-chip weight DMA is contiguous per partition
    wt = np.ascontiguousarray(wt.transpose(1, 0, 2))    # [128, 8, 128]
    wt2 = np.ascontiguousarray(wt2.transpose(1, 0, 2))  # [33, 8, 32]
    v0row = v0.reshape(1, HD).astype(np_cdtype)   # [1, 512]
    return wt, wt2, v0row


def _build_program(cdtype_name):
    import concourse.bass as bass
    import concourse.tile as tile
    from concourse import bacc, mybir

    cdtype = mybir.dt.bfloat16 if cdtype_name == "bf16" else mybir.dt.float32
    f32 = mybir.dt.float32

    nc = bacc.Bacc("TRN2", target_bir_lowering=False, debug=False,
                   num_devices=NCORES)

    x_d = nc.dram_tensor("x", [BPC, T, HD], f32, kind="ExternalInput").ap()
    wt_d = nc.dram_tensor("wt", [C + 1, H, C + 1], cdtype, kind="ExternalInput").ap()
    wt2_d = nc.dram_tensor("wt2", [REM + 1, H, REM], cdtype,
                           kind="ExternalInput").ap()
    v0_d = nc.dram_tensor("v0r", [1, HD], cdtype, kind="ExternalInput").ap()
    out_d = nc.dram_tensor("out", [BPC, T, HD], f32, kind="ExternalOutput").ap()

    from contextlib import ExitStack
    with tile.TileContext(nc) as tc, ExitStack() as ctx:
        consts = ctx.enter_context(tc.tile_pool(name="consts", bufs=1))
        in_pool = ctx.enter_context(tc.tile_pool(name="inp", bufs=6))
        in2_pool = ctx.enter_context(tc.tile_pool(name="inp2", bufs=2))
        out_pool = ctx.enter_context(tc.tile_pool(name="outp", bufs=6))
        out2_pool = ctx.enter_context(tc.tile_pool(name="outp2", bufs=2))
        psum_pool = ctx.enter_context(tc.tile_pool(name="psum", bufs=6,
                                                   space="PSUM"))
        psum2_pool = ctx.enter_context(tc.tile_pool(name="psum2", bufs=2,
                                                    space="PSUM"))
        if True:
            # --- constants ---
            wt_s = consts.tile([C + 1, H, C + 1], cdtype)   # [128, 8, 128]
            nc.sync.dma_start(out=wt_s[:], in_=wt_d)
            wt2_s = consts.tile([REM + 1, H, REM], cdtype)  # [33, 8, 32]
            nc.sync.dma_start(out=wt2_s[:], in_=wt2_d)
            v0_s = consts.tile([1, HD], cdtype)
            nc.sync.dma_start(out=v0_s[:], in_=v0_d[:])

            # --- tile handles ---
            in_tiles = {(b, g): in_pool.tile([C + 1, 4, HD], cdtype, tag="in", name=f"in_{b}_{g}")
                        for b in range(BPC) for g in range(GROUPS)}
            in2_tiles = {b: in2_pool.tile([REM + 1, HD], cdtype, tag="in2", name=f"in2_{b}")
                         for b in range(BPC)}
            stage_pool = ctx.enter_context(tc.tile_pool(name="stg", bufs=3))
            stage2_pool = ctx.enter_context(tc.tile_pool(name="stg2", bufs=2))

            def load_group(b, g):
                # Must be emitted BEFORE any carry copy that targets this
                # tile's row 0: the full-tile cast writes garbage there and
                # Tile orders same-region writes by program order.
                #
                # DMAs whose SBUF side is not aligned to the 8-partition
                # AXI port groups serialize onto a single SDMA engine
                # (measured 24 GB/s), so the load covers all 128 partitions
                # with an overlapping source AP: row 0 receives the chunk's
                # preceding time row (real data, replaced by the carry copy
                # before any matmul reads it).
                it = in_tiles[(b, g)]
                xb = x_d[b]
                stg = stage_pool.tile([C + 1, 4, HD], f32, tag="stg")
                if g > 0:
                    src = bass.AP(
                        tensor=xb.tensor,
                        offset=xb.offset + (4 * C * g - 1) * HD,
                        ap=[[HD, C + 1], [C * HD, 4], [1, HD]],
                    )
                    nc.sync.dma_start(out=stg[:, :, :], in_=src)
                else:
                    # chunks 1..3 via the overlap trick; chunk 0 has no
                    # predecessor row, so its x rows load misaligned (254KB
                    # once per batch, hidden in startup) plus one junk row.
                    src = bass.AP(
                        tensor=xb.tensor,
                        offset=xb.offset + (C - 1) * HD,
                        ap=[[HD, C + 1], [C * HD, 3], [1, HD]],
                    )
                    nc.sync.dma_start(out=stg[:, 1:4, :], in_=src)
                    nc.sync.dma_start(out=stg[1:C + 1, 0, :], in_=xb[0:C, :])
                    nc.sync.dma_start(out=stg[0:1, 0, :], in_=xb[0:1, :])
                cast_eng = [nc.gpsimd.tensor_copy, nc.gpsimd.tensor_copy,
                            nc.vector.tensor_copy, nc.scalar.copy][
                    (g * BPC + b) % 4]
                cast_eng(it[:, :, :], stg[:, :, :])
                if g == 0:
                    # carry-in for chunk 0 is v0
                    nc.vector.tensor_copy(it[0:1, 0, :], v0_s[:])

            for b in range(BPC):
                load_group(b, 0)
            for g in range(GROUPS):
                for b in range(BPC):
                    if g + 1 < GROUPS:
                        load_group(b, g + 1)
                    it = in_tiles[(b, g)]

                    ot = out_pool.tile([C, 4, HD], mybir.dt.float32, tag="out")
                    for k in range(4):
                        ps = psum_pool.tile([C + 1, HD], mybir.dt.float32, tag="ps")
                        for hh in range(H):
                            nc.tensor.matmul(
                                out=ps[:, hh * D:(hh + 1) * D],
                                lhsT=wt_s[:, hh, :],
                                rhs=it[:, k, hh * D:(hh + 1) * D],
                                start=True, stop=True,
                            )
                        # carry: corrected last row (at PSUM partition 0 via
                        # the lhsT permutation) -> next chunk's rhs row 0
                        if k < 3:
                            dst = in_tiles[(b, g)][0:1, k + 1, :]
                        elif g < GROUPS - 1:
                            dst = in_tiles[(b, g + 1)][0:1, 0, :]
                        else:
                            dst = in2_tiles[b][0:1, :]
                        nc.vector.tensor_copy(dst, ps[0:1, :])
                        # evict to f32 output staging (rows stay permuted)
                        nc.scalar.copy(ot[:, k, :], ps[0:C, :])
                    dstv = out_d[b, 4 * C * g: 4 * C * (g + 1), :] \
                        .rearrange("(k p) c -> p k c", p=C)
                    # un-permute: ot partition 0 = chunk's last time row
                    nc.sync.dma_start(out=dstv[0:C - 1, :, :],
                                      in_=ot[1:C, :, :])
                    nc.sync.dma_start(out=dstv[C - 1:C, :, :],
                                      in_=ot[0:1, :, :])

            # --- tail chunk (32 rows) per batch ---
            for b in range(BPC):
                it2 = in2_tiles[b]
                src = x_d[b, NFULL * C:, :]                  # [32, 512]
                if cdtype_name == "bf16":
                    nc.gpsimd.dma_start(out=it2[1:REM + 1, :], in_=src)
                else:
                    nc.sync.dma_start(out=it2[1:REM + 1, :], in_=src)
                ps2 = psum2_pool.tile([REM, HD], mybir.dt.float32, tag="ps2")
                for hh in range(H):
                    nc.tensor.matmul(
                        out=ps2[:, hh * D:(hh + 1) * D],
                        lhsT=wt2_s[:, hh, :],
                        rhs=it2[:, hh * D:(hh + 1) * D],
                        start=True, stop=True,
                    )
                ot2 = out2_pool.tile([REM, HD], mybir.dt.float32, tag="out2")
                nc.scalar.copy(ot2[:], ps2[:])
                nc.sync.dma_start(out=out_d[b, NFULL * C:, :], in_=ot2[:])

    nc.compile()
    return nc


def _get_program():
    key = COMPUTE_DTYPE
    if key not in _cache:
        _cache[key] = _build_program(key)
    return _cache[key]


def kernel(values, smoothing_weight, v0):
    import ml_dtypes
    from concourse.bass_utils import run_bass_kernel_spmd

    np_cdtype = ml_dtypes.bfloat16 if COMPUTE_DTYPE == "bf16" else np.float32
    wt, wt2, v0row = _host_constants(smoothing_weight, v0, np_cdtype)

    nc = _get_program()
    x = np.ascontiguousarray(values.astype(np.float32)
                             .reshape(B, T, HD))
    in_maps = []
    for core in range(NCORES):
        shard = np.ascontiguousarray(x[core * BPC:(core + 1) * BPC])
        in_maps.append({"x": shard, "wt": wt, "wt2": wt2, "v0r": v0row})

    res = run_bass_kernel_spmd(nc, in_maps, list(range(NCORES)))
    outs = [res.results[i]["out"].reshape(BPC, T, H, D)
            for i in range(NCORES)]
    return np.concatenate(outs, axis=0).astype(np.float32)


# revision 16
# speedup vs baseline: 5.2296x; 1.1600x over previous
"""Trainium2 Bass kernel for exponential smoothing (EMA over time).

Math: out[b,t,h,d] = w_h^{t+1} v0[h,d] + sum_{j<=t} (1-w_h) w_h^{t-j} x[b,j,h,d]
(w = sigmoid(smoothing_weight)), i.e. the scan s_t = w s_{t-1} + (1-w) x_t with
s_{-1} = v0.

Kernel strategy (per core, data-parallel over batch: 16 batches / 8 cores):
  - Time is processed in chunks of C=127. Each chunk is one set of 8 per-head
    matmuls: rhs = [carry_row; x rows] (128 x 64 per head), lhsT packs the
    decay column w^{p+1} (for the carry) on top of the lower-triangular
    smoothing weights (1-w) w^{p-j}. The *corrected* last output row of a
    chunk IS the carry for the next chunk, so cross-chunk propagation is a
    single [1,512] PSUM->SBUF row copy per chunk.
  - 4096 = 32*127 + 32: 32 full chunks + one 32-row tail chunk per batch.
  - Input tiles are loaded contiguously ([128, 4*512] per 4-chunk group,
    ~1MB per dma_start) with an f32->bf16 cast on the SWDGE path; matmuls run
    in bf16 (fp32 PSUM accumulate), output is evicted to f32 and stored
    contiguously.
"""

import numpy as np

B, T, H, D = 16, 4096, 8, 64
HD = H * D                    # 512
C = 127                       # chunk length (1 row reserved for the carry)
NFULL = T // C                # 32 full chunks
REM = T - NFULL * C           # 32-row tail chunk
GROUPS = NFULL // 4           # 8 groups of 4 chunks (one ~1MB DMA each)
NCORES = 8
BPC = B // NCORES             # batches per core

COMPUTE_DTYPE = "bf16"        # "bf16" | "fp32"

_cache = {}


def _host_constants(smoothing_weight, v0, np_cdtype):
    """Parameter-derived constants, computed in fp64 on host."""
    w = 1.0 / (1.0 + np.exp(-smoothing_weight.astype(np.float64)))  # [H,1]
    w = w[:, 0]

    def make_lhsT(n):
        # [H, n+1, n]; row 0 = w^(p+1) (carry decay), row 1+j = (1-w) w^(p-j)
        lt = np.zeros((H, n + 1, n), dtype=np.float64)
        p = np.arange(n)
        for hh in range(H):
            lt[hh, 0, :] = w[hh] ** (p + 1)
            for j in range(n):
                lt[hh, 1 + j, j:] = (1.0 - w[hh]) * w[hh] ** (p[j:] - j)
        return lt.astype(np_cdtype)

    wt = make_lhsT(C)          # [H, 128, 127]
    # Engine APs must start at 32-aligned partitions, so the carry row (last
    # output row of a chunk) is moved to PSUM partition 0: permute lhsT
    # columns to [last, 0..last-1]; the out-DMA un-permutes.
    wt = np.concatenate([wt[:, :, C - 1:], wt[:, :, :C - 1]], axis=2)
    wt2 = make_lhsT(REM)       # [H, 33, 32] (tail: no carry out, unpermuted)
    # pad M to 128 (zero column): Fast Weight Load needs NumWeights == 128;
    # the extra PSUM row is never read
    wt = np.concatenate([wt, np.zeros((H, C + 1, 1), wt.dtype)], axis=2)
    # [K, H, M] layout so the on-chip weight DMA is contiguous per partition
    wt = np.ascontiguousarray(wt.transpose(1, 0, 2))    # [128, 8, 128]
    wt2 = np.ascontiguousarray(wt2.transpose(1, 0, 2))  # [33, 8, 32]
    v0row = v0.reshape(1, HD).astype(np_cdtype)   # [1, 512]
    return wt, wt2, v0row


def _build_program(cdtype_name):
    import concourse.bass as bass
    import concourse.tile as tile
    from concourse import bacc, mybir

    cdtype = mybir.dt.bfloat16 if cdtype_name == "bf16" else mybir.dt.float32
    f32 = mybir.dt.float32

    nc = bacc.Bacc("TRN2", target_bir_lowering=False, debug=False,
                   num_devices=NCORES)

    x_d = nc.dram_tensor("x", [BPC, T, HD], f32, kind="ExternalInput").ap()
    wt_d = nc.dram_tensor("wt", [C + 1, H, C + 1], cdtype, kind="ExternalInput").ap()
    wt2_d = nc.dram_tensor("wt2", [REM + 1, H, REM], cdtype,
                           kind="ExternalInput").ap()
    v0_d = nc.dram_tensor("v0r", [1, HD], cdtype, kind="ExternalInput").ap()
    out_d = nc.dram_tensor("out", [BPC, T, HD], f32, kind="ExternalOutput").ap()

    from contextlib import ExitStack
    with tile.TileContext(nc) as tc, ExitStack() as ctx:
        consts = ctx.enter_context(tc.tile_pool(name="consts", bufs=1))
        in_pool = ctx.enter_context(tc.tile_pool(name="inp", bufs=8))
        in2_pool = ctx.enter_context(tc.tile_pool(name="inp2", bufs=2))
        out_pool = ctx.enter_context(tc.tile_pool(name="outp", bufs=6))
        out2_pool = ctx.enter_context(tc.tile_pool(name="outp2", bufs=2))
        psum_pool = ctx.enter_context(tc.tile_pool(name="psum", bufs=6,
                                                   space="PSUM"))
        psum2_pool = ctx.enter_context(tc.tile_pool(name="psum2", bufs=2,
                                                    space="PSUM"))
        if True:
            # --- constants ---
            wt_s = consts.tile([C + 1, H, C + 1], cdtype)   # [128, 8, 128]
            nc.sync.dma_start(out=wt_s[:], in_=wt_d)
            wt2_s = consts.tile([REM + 1, H, REM], cdtype)  # [33, 8, 32]
            nc.sync.dma_start(out=wt2_s[:], in_=wt2_d)
            v0_s = consts.tile([1, HD], cdtype)
            nc.sync.dma_start(out=v0_s[:], in_=v0_d[:])

            # --- tile handles ---
            in_tiles = {(b, g): in_pool.tile([C + 1, 4, HD], cdtype, tag="in", name=f"in_{b}_{g}")
                        for b in range(BPC) for g in range(GROUPS)}
            in2_tiles = {b: in2_pool.tile([REM + 1, HD], cdtype, tag="in2", name=f"in2_{b}")
                         for b in range(BPC)}
            stage_pool = ctx.enter_context(tc.tile_pool(name="stg", bufs=4))
            stage2_pool = ctx.enter_context(tc.tile_pool(name="stg2", bufs=2))

            def load_group(b, g):
                # Must be emitted BEFORE any carry copy that targets this
                # tile's row 0: the full-tile cast writes garbage there and
                # Tile orders same-region writes by program order.
                #
                # DMAs whose SBUF side is not aligned to the 8-partition
                # AXI port groups serialize onto a single SDMA engine
                # (measured 24 GB/s), so the load covers all 128 partitions
                # with an overlapping source AP: row 0 receives the chunk's
                # preceding time row (real data, replaced by the carry copy
                # before any matmul reads it).
                it = in_tiles[(b, g)]
                xb = x_d[b]
                stg = stage_pool.tile([C + 1, 4, HD], f32, tag="stg")
                if g > 0:
                    src = bass.AP(
                        tensor=xb.tensor,
                        offset=xb.offset + (4 * C * g - 1) * HD,
                        ap=[[HD, C + 1], [C * HD, 4], [1, HD]],
                    )
                    nc.sync.dma_start(out=stg[:, :, :], in_=src)
                else:
                    # chunks 1..3 via the overlap trick; chunk 0 has no
                    # predecessor row, so its x rows load misaligned (254KB
                    # once per batch, hidden in startup) plus one junk row.
                    src = bass.AP(
                        tensor=xb.tensor,
                        offset=xb.offset + (C - 1) * HD,
                        ap=[[HD, C + 1], [C * HD, 3], [1, HD]],
                    )
                    nc.sync.dma_start(out=stg[:, 1:4, :], in_=src)
                    nc.sync.dma_start(out=stg[1:C + 1, 0, :], in_=xb[0:C, :])
                    nc.sync.dma_start(out=stg[0:1, 0, :], in_=xb[0:1, :])
                cast_eng = [nc.gpsimd.tensor_copy, nc.scalar.copy][
                    (g * BPC + b) % 2]
                cast_eng(it[:, :, :], stg[:, :, :])
                if g == 0:
                    # carry-in for chunk 0 is v0
                    nc.vector.tensor_copy(it[0:1, 0, :], v0_s[:])

            for b in range(BPC):
                load_group(b, 0)
            for b in range(BPC):
                load_group(b, 1)
            for g in range(GROUPS):
                for b in range(BPC):
                    if g + 2 < GROUPS:
                        load_group(b, g + 2)
                    it = in_tiles[(b, g)]

                    ot = out_pool.tile([C, 4, HD], mybir.dt.float32, tag="out")
                    for k in range(4):
                        ps = psum_pool.tile([C + 1, HD], mybir.dt.float32, tag="ps")
                        for hh in range(H):
                            nc.tensor.matmul(
                                out=ps[:, hh * D:(hh + 1) * D],
                                lhsT=wt_s[:, hh, :],
                                rhs=it[:, k, hh * D:(hh + 1) * D],
                                start=True, stop=True,
                            )
                        # carry: corrected last row (at PSUM partition 0 via
                        # the lhsT permutation) -> next chunk's rhs row 0
                        if k < 3:
                            dst = in_tiles[(b, g)][0:1, k + 1, :]
                        elif g < GROUPS - 1:
                            dst = in_tiles[(b, g + 1)][0:1, 0, :]
                        else:
                            dst = in2_tiles[b][0:1, :]
                        nc.vector.tensor_copy(dst, ps[0:1, :])
                        # evict to f32 output staging (rows stay permuted)
                        nc.scalar.copy(ot[:, k, :], ps[0:C, :])
                    dstv = out_d[b, 4 * C * g: 4 * C * (g + 1), :] \
                        .rearrange("(k p) c -> p k c", p=C)
                    # un-permute: ot partition 0 = chunk's last time row
                    nc.sync.dma_start(out=dstv[0:C - 1, :, :],
                                      in_=ot[1:C, :, :])
                    nc.sync.dma_start(out=dstv[C - 1:C, :, :],
                                      in_=ot[0:1, :, :])

            # --- tail chunk (32 rows) per batch ---
            for b in range(BPC):
                it2 = in2_tiles[b]
                src = x_d[b, NFULL * C:, :]                  # [32, 512]
                if cdtype_name == "bf16":
                    nc.gpsimd.dma_start(out=it2[1:REM + 1, :], in_=src)
                else:
                    nc.sync.dma_start(out=it2[1:REM + 1, :], in_=src)
                ps2 = psum2_pool.tile([REM, HD], mybir.dt.float32, tag="ps2")
                for hh in range(H):
                    nc.tensor.matmul(
                        out=ps2[:, hh * D:(hh + 1) * D],
                        lhsT=wt2_s[:, hh, :],
                        rhs=it2[:, hh * D:(hh + 1) * D],
                        start=True, stop=True,
                    )
                ot2 = out2_pool.tile([REM, HD], mybir.dt.float32, tag="out2")
                nc.scalar.copy(ot2[:], ps2[:])
                nc.sync.dma_start(out=out_d[b, NFULL * C:, :], in_=ot2[:])

    nc.compile()
    return nc


def _get_program():
    key = COMPUTE_DTYPE
    if key not in _cache:
        _cache[key] = _build_program(key)
    return _cache[key]


def kernel(values, smoothing_weight, v0):
    import ml_dtypes
    from concourse.bass_utils import run_bass_kernel_spmd

    np_cdtype = ml_dtypes.bfloat16 if COMPUTE_DTYPE == "bf16" else np.float32
    wt, wt2, v0row = _host_constants(smoothing_weight, v0, np_cdtype)

    nc = _get_program()
    x = np.ascontiguousarray(values.astype(np.float32)
                             .reshape(B, T, HD))
    in_maps = []
    for core in range(NCORES):
        shard = np.ascontiguousarray(x[core * BPC:(core + 1) * BPC])
        in_maps.append({"x": shard, "wt": wt, "wt2": wt2, "v0r": v0row})

    res = run_bass_kernel_spmd(nc, in_maps, list(range(NCORES)))
    outs = [res.results[i]["out"].reshape(BPC, T, H, D)
            for i in range(NCORES)]
    return np.concatenate(outs, axis=0).astype(np.float32)
